# revision 20
# baseline (speedup 1.0000x reference)
"""Trainium2 Bass kernel for nn_CrossAttention (B=2, N=2048, D=1024, H=16).

Token-sharded design (8 cores): core c -> (batch b = c//4, q-slice g = c%4).
Each core owns 512 query tokens of one batch and computes ALL 16 heads for
them, so its output [512, 1024] is complete locally - no output collective.

The K projection is REPLICATED on every core (full [1024, 2048] K^T, +41us
of PE work) - cheaper than a K AllGather whose mesh setup + serialization
with the V gather cost ~120us of latency.  Only V is gathered: each core
projects its 4-head V slice [2048, 256] and AllGathers 1MB/rank (bf16)
within its batch group, overlapped with the K/Q projections and
the score/exp half of attention (PV matmuls stall briefly; PE catches up).

All matmuls run in bf16 with fp32 PSUM accumulation.  (fp8 was tried for
the PV path and costs ~3% relative error: per-element quantization noise
of P or V lands ~1:1 on the attention output - it is itself an average of
the same scale as the noise average - so fp8 is unusable here.)

Attention per head pair (even head on PE rows 0-63, odd on 64-127 via
tile_position): scores^T for both heads land in one 2-bank [128, 1024]
PSUM tile, one ACT Exp instruction covers the pair, two PV matmuls
with ones-augmented V accumulate x^T plus the softmax denominators.
K-proj/Q-proj/attention are interleaved per head pair ("waves") so ACT
(the bottleneck engine, ~137us of Exp) is fed from ~40us onward.

Normalization without long DVE reciprocals: denominators bounce
PSUM->DRAM->[128,8] SBUF, one cheap 8-elem/lane reciprocal, bounce back to
a [128, 512] broadcast tile (even-head halves on partitions 0-63, odd on
64-127), then a single in-place DVE multiply on xt.
"""

import numpy as np

B = 2
NT = 2048
D = 1024
HEADS = 16
DH = 64
NCORES = 8
CPB = 4            # cores per batch
QSL = NT // CPB    # 512 own query tokens per core
HG = HEADS // CPB  # 4 heads per V shard group
HGD = HG * DH      # 256 projection dims per group
VW = DH + 1        # V columns per head incl. ones column
GROUPS = [[0, 1, 2, 3], [4, 5, 6, 7]]
SCALE = DH ** -0.5
LN64 = float(np.log(64.0))
NKT = NT // 128    # 16 key-token tiles

_patched = False


def _patch_tile_drain():
    """This container's walrus rejects >1 sync-wait on a Drain
    (CoreV3GenImpl setupSyncWait<CTRL_NO_STRUCT>: "Too many sync wait
    commands").  Split the final TileContext drain's waits across a chain
    of single-wait drains; semaphores are monotonic so sequential waits
    are equivalent to one multi-wait."""
    global _patched
    if _patched:
        return
    import concourse.tile as tile
    import concourse.mybir as mybir
    from concourse.vector_clock import ScopedClock

    _uid = [0]

    def _split_multiwaits(nc):
        for f in nc.m.functions:
            for bb in f.blocks:
                il = bb.instructions
                i = 0
                while i < len(il):
                    inst = il[i]
                    si = inst.sync_info
                    if si is not None and len(si.on_wait) > 1:
                        waits = list(si.on_wait)
                        inst.sync_info = mybir.SyncInfo(
                            on_wait=[waits[-1]], on_update=list(si.on_update)
                        )
                        for w in waits[:-1]:
                            _uid[0] += 1
                            nop = mybir.InstEventSemaphore(
                                name=f"WSPLIT-{_uid[0]}",
                                engine=inst.engine,
                                ins=[],
                                outs=[],
                                sync_info=mybir.SyncInfo(
                                    on_wait=[w], on_update=[]),
                            )
                            il.insert(i, nop)
                            i += 1
                    i += 1

    def _drain_and_barrier(self, tick_clock, wait_clock):
        nc = self.nc
        drain_inst = nc.sync.drain()
        wait_clock.add_sem_waits(
            drain_inst.ins, ScopedClock({None: tick_clock.global_clock})
        )
        si = drain_inst.ins.sync_info
        if si is not None and len(si.on_wait) > 1:
            waits = list(si.on_wait)
            drain_inst.ins.sync_info = mybir.SyncInfo(
                on_wait=[waits[0]], on_update=list(si.on_update)
            )
            for w in waits[1:]:
                extra = nc.sync.drain()
                extra.ins.sync_info = mybir.SyncInfo(on_wait=[w], on_update=[])

        _split_multiwaits(nc)
        nc.all_engine_barrier()
        assert self.sems is not None
        popped = nc._tile_sem_poison_stack.pop()
        assert popped is self._sem_poison
        nc.clear_and_free_semaphores(list(self.sems.allocated().values()))
        nc.all_engine_barrier()

    tile.TileContext._drain_and_barrier = _drain_and_barrier
    _patched = True


def build_program():
    """Build the SPMD Bass program (one NeuronCore's view)."""
    _patch_tile_drain()
    import concourse.bass as bass
    import concourse.tile as tile
    import concourse.mybir as mybir

    f32 = mybir.dt.float32
    bf16 = mybir.dt.bfloat16
    fp8 = mybir.dt.float8e4
    EXP = mybir.ActivationFunctionType.Exp

    nc = bass.Bass("TRN2", target_bir_lowering=False, debug=False,
                   num_devices=NCORES)

    # pre-shaped host-side: [128, 8, X] with row (t p) -> [p, t, :]
    xq3 = nc.dram_tensor("xq3", [128, 8, QSL], bf16, kind="ExternalInput")
    xk3 = nc.dram_tensor("xk3", [128, 8, NT], bf16, kind="ExternalInput")
    xv3 = nc.dram_tensor("xv3", [128, 8, NT], bf16, kind="ExternalInput")
    wq3 = nc.dram_tensor("wq3", [128, 8, D], bf16, kind="ExternalInput")
    wk3 = nc.dram_tensor("wk3", [128, 8, D], bf16, kind="ExternalInput")
    wv3 = nc.dram_tensor("wv3", [128, 8, D], bf16, kind="ExternalInput")
    wo3 = nc.dram_tensor("wo3", [128, 8, D], bf16, kind="ExternalInput")
    bo = nc.dram_tensor("bo", [D], f32, kind="ExternalInput")
    out = nc.dram_tensor("out", [QSL, D], f32, kind="ExternalOutput")

    rbounce = nc.dram_tensor("rbounce", [4, 2 * QSL], f32)
    rbounce2 = nc.dram_tensor("rbounce2", [4, 2 * QSL], f32)

    with tile.TileContext(nc) as tc:
        from contextlib import ExitStack
        with ExitStack() as ctx:
            const = ctx.enter_context(tc.tile_pool(name="const", bufs=1))
            persist = ctx.enter_context(tc.tile_pool(name="persist", bufs=1))
            pt_pool = ctx.enter_context(tc.tile_pool(name="pt", bufs=5))
            misc = ctx.enter_context(tc.tile_pool(name="misc", bufs=4))
            nrm = ctx.enter_context(tc.tile_pool(name="nrm", bufs=2))
            outsb = ctx.enter_context(tc.tile_pool(name="outsb", bufs=2))
            # PSUM: 8 banks of [128, 512]f32.  big_ps: 2-bank [128, 1024]
            # tiles (score pairs, K/Q/out-proj accs); xa_ps: 1-bank tiles
            # (attention x^T accs, V-proj accs).
            kt_pool = ctx.enter_context(tc.tile_pool(name="ktp", bufs=2))
            big_ps = ctx.enter_context(
                tc.tile_pool(name="big_ps", bufs=2, space="PSUM"))
            xa_ps = ctx.enter_context(
                tc.tile_pool(name="xa_ps", bufs=4, space="PSUM"))

            # --- warm the Exp activation table during the DMA ramp --------
            junk = const.tile([1, 8], f32)
            nc.vector.memset(junk[:], 0.0)
            jout = const.tile([1, 8], bf16)
            nc.scalar.activation(jout[:], junk[:], EXP)

            # --- V-proj constants first (V gather is the long pole) -------
            wv_sb = const.tile([128, 8, D], bf16)
            for k in range(8):
                nc.sync.dma_start(out=wv_sb[:, k, :], in_=wv3[:, k, :])

            # --- persistent tiles -----------------------------------------
            xk_sb = persist.tile([128, 8, NT], bf16)   # X_k^T (K-proj rhs)
            qt_sb = persist.tile([128, 8, QSL], bf16)
            v_sb = persist.tile([128, NKT, HEADS * VW], bf16)
            xt_sb = persist.tile([128, 8, QSL], bf16)  # x^T normalized
            psb = persist.tile([128, 8, 512], bf16)    # out-proj partials

            nc.vector.memset(
                v_sb[:].rearrange("p m (h c) -> p (m h) c", c=VW)[:, :, DH:],
                1.0)

            # --- V projection (ALL 16 heads, replicated; no collective) ---
            # V[tok, vcol] = sum_d X_v^T[d, tok] Wv^T[d, vcol]
            # X_v^T is staged through xk_sb (xk3 overwrites it afterwards).
            for k in range(8):
                nc.sync.dma_start(out=xk_sb[:, k, :], in_=xv3[:, k, :])
            for mg in range(4):
                for m in range(4):
                    mt = 4 * mg + m
                    acc = big_ps.tile([128, 1024], f32, tag="mm",
                                      name="vacc")
                    for k in range(8):
                        for h in range(2):
                            nc.tensor.matmul(
                                acc[:, 512 * h:512 * (h + 1)],
                                xk_sb[:, k, 128 * mt:128 * (mt + 1)],
                                wv_sb[:, k, 512 * h:512 * (h + 1)],
                                start=(k == 0), stop=(k == 7))
                    nc.vector.tensor_copy(
                        v_sb[:, mt, :].rearrange(
                            "p (h c) -> p h c", c=VW)[:, :, 0:DH],
                        acc[:, 0:1024].rearrange(
                            "p (h c) -> p h c", c=DH))

            # xk3 overwrites the X_v^T staging once V-proj has read it
            for k in range(8):
                nc.sync.dma_start(out=xk_sb[:, k, :], in_=xk3[:, k, :])

            # --- remaining constants (queued behind V-proj traffic) -------
            wk_sb = const.tile([128, 8, D], bf16)
            wq_sb = const.tile([128, 8, D], bf16)
            wo_sb = const.tile([128, 8, D], bf16)
            bias_sb = const.tile([128, D], f32)
            xq_sb = const.tile([128, 8, QSL], bf16)
            nc.sync.dma_start(out=wk_sb[:], in_=wk3[:])
            nc.sync.dma_start(out=xq_sb[:], in_=xq3[:])
            nc.sync.dma_start(out=wq_sb[:], in_=wq3[:])


            # --- waves: K-proj block w, Q-proj block w, attention pair w --
            for w in range(8):
                # K^T block w: KT[128w+p, tok] = sum_d Wk^T[d, .] Xk^T[d, .]
                kt_w = kt_pool.tile([128, NT], bf16)
                for n in range(4):
                    nsl = slice(512 * n, 512 * (n + 1))
                    acc = big_ps.tile([128, 1024], f32, tag="mm", name="kacc")
                    for k in range(8):
                        nc.tensor.matmul(
                            acc[:, 0:512],
                            wk_sb[:, k, 128 * w:128 * (w + 1)],
                            xk_sb[:, k, nsl],
                            start=(k == 0), stop=(k == 7))
                    nc.vector.tensor_copy(kt_w[:, nsl], acc[:, 0:512])
                # Q^T block w
                acc = big_ps.tile([128, 1024], f32, tag="mm", name="qacc")
                for k in range(8):
                    nc.tensor.matmul(
                        acc[:, 0:512],
                        wq_sb[:, k, 128 * w:128 * (w + 1)],
                        xq_sb[:, k, :],
                        start=(k == 0), stop=(k == 7))
                nc.vector.tensor_copy(qt_sb[:, w, :], acc[:, 0:512])

                # attention for head pair w
                he, ho = 2 * w, 2 * w + 1
                xa_e = xa_ps.tile([VW, 512], f32, tag="xa", name="xa")
                xa_o = xa_ps.tile([VW, 512], f32, tag="xa", name="xa")
                for kt in range(NKT):
                    ksl = slice(128 * kt, 128 * (kt + 1))
                    st = big_ps.tile([128, 1024], f32, tag="mm", name="st")
                    nc.tensor.matmul(
                        st[:, 0:512],
                        kt_w[0:64, ksl], qt_sb[0:64, w, :],
                        tile_position=(0, 0))
                    nc.tensor.matmul(
                        st[:, 512:1024],
                        kt_w[64:128, ksl], qt_sb[64:128, w, :],
                        tile_position=(64, 0))
                    pt = pt_pool.tile([128, 1024], bf16)
                    nc.scalar.activation(pt[:], st[:], EXP, scale=SCALE)
                    nc.tensor.matmul(
                        xa_e[:], v_sb[:, kt, VW * he:VW * (he + 1)],
                        pt[:, 0:512],
                        start=(kt == 0), stop=(kt == NKT - 1))
                    nc.tensor.matmul(
                        xa_o[:], v_sb[:, kt, VW * ho:VW * (ho + 1)],
                        pt[:, 512:1024],
                        start=(kt == 0), stop=(kt == NKT - 1))

                # extract x (unnormalized) and denominators; free PSUM fast
                nc.vector.tensor_copy(xt_sb[0:DH, w, :], xa_e[0:DH, :])
                tm = misc.tile([DH, 512], bf16, tag="tm")
                nc.vector.tensor_copy(tm[:], xa_o[0:DH, :])
                nc.sync.dma_start(out=xt_sb[DH:128, w, :], in_=tm[:])
                den = nrm.tile([1, 2 * QSL], f32, tag="den")
                nc.vector.tensor_copy(den[:, 0:512], xa_e[DH:VW, :])
                nc.vector.tensor_copy(den[:, 512:1024], xa_o[DH:VW, :])
                rb = rbounce[w % 4:w % 4 + 1, :]
                nc.sync.dma_start(out=rb, in_=den[:])
                # reciprocal at [128, 8] (8 elems/lane) via DRAM reshape
                rsh = misc.tile([128, 8], f32, tag="rsh")
                nc.sync.dma_start(
                    out=rsh[:], in_=rb.rearrange("r (p e) -> (r p) e", p=128))
                rsh2 = misc.tile([128, 8], f32, tag="rsh2")
                nc.vector.reciprocal(rsh2[:], rsh[:])
                rb2 = rbounce2[w % 4:w % 4 + 1, :]
                nc.sync.dma_start(
                    out=rb2.rearrange("r (p e) -> (r p) e", p=128),
                    in_=rsh2[:])
                bc = nrm.tile([128, 512], f32, tag="bc")
                nc.sync.dma_start(
                    out=bc[0:64, :], in_=rb2[:, 0:512].partition_broadcast(64))
                nc.sync.dma_start(
                    out=bc[64:128, :],
                    in_=rb2[:, 512:1024].partition_broadcast(64))
                nc.vector.tensor_mul(
                    xt_sb[:, w, :], xt_sb[:, w, :], bc[:])

            nc.sync.dma_start(out=wo_sb[:], in_=wo3[:])
            nc.sync.dma_start(out=bias_sb[:],
                              in_=bo[:].partition_broadcast(128))

            # --- output projection + bias ---------------------------------
            # out[tok, o] = sum_x x^T[x, tok] Wo^T[x, o] + bo[o]
            # Partials over x-blocks 0..6 (+bias) run in freed xa_ps slots
            # while the last head pair normalizes; the final pass per tile
            # is one matmul (x-block 7) + one DVE add.
            for m in range(4):
                tsl = slice(128 * m, 128 * (m + 1))
                for n in range(2):
                    osl = slice(512 * n, 512 * (n + 1))
                    acc = xa_ps.tile([128, 512], f32, tag="xa", name="opp")
                    for k in range(7):
                        nc.tensor.matmul(
                            acc[:],
                            xt_sb[:, k, tsl],
                            wo_sb[:, k, osl],
                            start=(k == 0), stop=(k == 6))
                    nc.vector.tensor_add(psb[:, 2 * m + n, :],
                                         acc[:], bias_sb[:, osl])
            for m in range(4):
                tsl = slice(128 * m, 128 * (m + 1))
                for n in range(2):
                    osl = slice(512 * n, 512 * (n + 1))
                    acc = xa_ps.tile([128, 512], f32, tag="xa", name="opf")
                    nc.tensor.matmul(
                        acc[:], xt_sb[:, 7, tsl], wo_sb[:, 7, osl])
                    ob = outsb.tile([128, 512], f32)
                    nc.vector.tensor_add(ob[:], acc[:],
                                         psb[:, 2 * m + n, :])
                    nc.sync.dma_start(out=out[tsl, osl], in_=ob[:])

    return nc


_CACHE = {}


def _get_program():
    if "nc" not in _CACHE:
        _CACHE["nc"] = build_program()
    return _CACHE["nc"]


def _pre3(mat_t):
    """[D, X] (row = (t p)) -> contiguous [128, 8, X]."""
    x = mat_t.shape[1]
    return np.ascontiguousarray(
        mat_t.reshape(8, 128, x).transpose(1, 0, 2))


def make_in_maps(query, key, value, Wq, Wk, Wv, Wo, bo):
    """Host-side sharding: per-core input dicts (bf16)."""
    import ml_dtypes
    bf = ml_dtypes.bfloat16

    def b(x):
        return np.asarray(x, dtype=np.float32).astype(bf)

    query = np.asarray(query, dtype=np.float32)
    wq3 = _pre3(b(np.asarray(Wq, dtype=np.float32).T))
    wk3 = _pre3(b(np.asarray(Wk, dtype=np.float32).T))
    wo3 = _pre3(b(np.asarray(Wo, dtype=np.float32).T))
    bo32 = np.ascontiguousarray(np.asarray(bo, dtype=np.float32))
    xk3 = [_pre3(b(np.asarray(key, dtype=np.float32)[bb].T))
           for bb in range(B)]
    xv3 = [_pre3(b(np.asarray(value, dtype=np.float32)[bb].T))
           for bb in range(B)]
    wv3 = _pre3(b(np.asarray(Wv, dtype=np.float32).T))

    in_maps = []
    for c in range(NCORES):
        bb, g = divmod(c, CPB)
        in_maps.append({
            "xq3": _pre3(b(query[bb, QSL * g:QSL * (g + 1), :].T)),
            "xk3": xk3[bb],
            "xv3": xv3[bb],
            "wq3": wq3,
            "wk3": wk3,
            "wv3": wv3,
            "wo3": wo3,
            "bo": bo32,
        })
    return in_maps


def assemble(results):
    """Concatenate per-core token slices into [B, NT, D]."""
    out = np.empty((B, NT, D), dtype=np.float32)
    for c in range(NCORES):
        bb, g = divmod(c, CPB)
        out[bb, QSL * g:QSL * (g + 1), :] = results[c]["out"]
    return out


def run(query, key, value, Wq, Wk, Wv, Wo, bo, trace=False):
    from concourse.bass_utils import run_bass_kernel_spmd
    nc = _get_program()
    in_maps = make_in_maps(query, key, value, Wq, Wk, Wv, Wo, bo)
    res = run_bass_kernel_spmd(nc, in_maps, core_ids=list(range(NCORES)),
                               trace=trace)
    return assemble(res.results), res


def kernel(query, key, value, qpos=None, kpos=None, Wq=None, Wk=None,
           Wv=None, Wo=None, bo=None):
    out, _ = run(query, key, value, Wq, Wk, Wv, Wo, bo)
    return out
